# revision 18
# baseline (speedup 1.0000x reference)
"""Trainium2 Bass kernel for BasicLSTM (B=64, T=512, D=U=512).

Sharding: data-parallel over batch across 8 cores (8 rows/core), weights
replicated; the sequential time scan runs locally per core.

Per-core strategy (everything unit-major / "transposed", all-SBUF):
  zx.T = Wk.T @ x.T + b is computed in 64-step chunks that OVERLAP the
  recurrent scan: while the scan consumes chunk c, the input projection
  for chunk c+1 streams in (DMA loads + bf16 casts on Pool + DMA-xbar
  transposes) and its matmuls/copy-outs slot into the PE/ACT/DVE idle
  windows of the scan steps.  zx.T lives in SBUF bf16, t-major
  (col = t*128 + a*32 + q*8 + b), so each scan step reads one contiguous
  128-col block and phase-A chunks write disjoint ranges.
  The g-gate columns of Wk/Wr/b are pre-scaled by 2 so the scan evaluates
  all four gates with a single sigmoid (tanh(x) = 2*sigm(2x)-1).

  Scan step (per core, batch 8, unit-major):
    psum[half] = zx.T[t] (identity matmul) + sum_k Wr[k,m].T @ h.T[k]
    s = sigmoid(psum)            (one ACT op per half)
    t2 = (s_g - 0.5) * s_i ; c' = 2*t2 + f*c   (fused DVE STT ops)
    h' = s_o * tanh(c')          (ACT + DVE)
  Psum half 1's 32 matmuls are issued first so its sigmoid fires
  mid-burst and its tail overlaps the half-0 matmuls; kk in {2,3}
  (reading h half 1, which each tail finishes first) precede kk in {0,1}.
  h is bf16 (feeds the next matmul), c stays fp32.
"""

import numpy as np

B, T, D, U = 64, 512, 512, 512
G = 4 * U            # gates
P = 128              # partitions
N_CORES = 8
B_LOC = B // N_CORES  # 8
KD = D // P          # 4 k-tiles for x@Wk
KU = U // P          # 4 k-tiles for h@Wr
M = G // P           # 16 m-tiles of gates
TC = 64              # timesteps per phase-A chunk
NCH = T // TC        # 8 chunks
FB = M * B_LOC       # 128 free cols of z per step
HB = FB // 2         # 64 cols per half

# natural gate order [i, f, g, o]; a = m//4 is the gate class
# halves: half h holds m-tiles {4a + q : a in 0..3} for q in {2h, 2h+1}
HALF_MS = [[0, 4, 8, 12, 1, 5, 9, 13], [2, 6, 10, 14, 3, 7, 11, 15]]
# the critical chain needs only i,f,g (classes 0..2); o feeds just h'=o*tanh(c)
IFG_MS = [[m for m in ms if m < 12] for ms in HALF_MS]
O_MS = [[m for m in ms if m >= 12] for ms in HALF_MS]

_CACHE = {}


def _build(time_steps=T):
    import concourse.bacc as bacc
    import concourse.tile as tile
    import concourse.mybir as mybir
    from concourse import masks
    from concourse import masks
    from concourse.alu_op_type import AluOpType
    from bass_rust import add_dep_helper

    f32 = mybir.dt.float32
    bf16 = mybir.dt.bfloat16
    AF = mybir.ActivationFunctionType

    nc = bacc.Bacc(
        "TRN2",
        target_bir_lowering=False,
        debug=False,
        enable_asserts=True,
        num_devices=N_CORES,
    )

    x_h = nc.dram_tensor("x", [B_LOC, T, D], f32, kind="ExternalInput")
    wk_h = nc.dram_tensor("Wk", [D, G], f32, kind="ExternalInput")
    wr_h = nc.dram_tensor("Wr", [U, G], f32, kind="ExternalInput")
    b_h = nc.dram_tensor("b", [G], f32, kind="ExternalInput")
    out_h = nc.dram_tensor("h_last", [B_LOC, U], f32, kind="ExternalOutput")

    x_ap = x_h.ap()

    def load_weight_bf16(dst, src_h, stage_pool):
        """[512, 2048] fp32 weight -> dst bf16 [128, 64*128] laid out as
        (k, new_m) tiles of [128, 128] with the [i,f,o,g] gate reorder.
        The g tiles (new m 12..15) are scaled by 2 for the sigmoid-only
        gate evaluation."""
        for k in range(KD):
            st = stage_pool.tile([P, G], f32, name="wstage", tag="wstage")
            nc.sync.dma_start(st[:], src_h.ap()[k * P:(k + 1) * P, :])
            # split the big casts between DVE and Pool so the prolog isn't
            # serialized on one engine
            for eng, nm0, w in ((nc.vector, 0, 4), (nc.gpsimd, 4, 4),
                                (nc.vector, 12, 2), (nc.gpsimd, 14, 2)):
                eng.tensor_copy(
                    dst[:, (k * M + nm0) * P:(k * M + nm0 + w) * P],
                    st[:, nm0 * P:(nm0 + w) * P],
                )
            nc.vector.tensor_scalar_mul(
                dst[:, (k * M + 8) * P:(k * M + 10) * P],
                st[:, 8 * P:10 * P],
                2.0,
            )
            nc.gpsimd.tensor_scalar_mul(
                dst[:, (k * M + 10) * P:(k * M + 12) * P],
                st[:, 10 * P:12 * P],
                2.0,
            )

    with tile.TileContext(nc) as tc:
        with (
            tc.tile_pool(name="persist", bufs=1) as persist_pool,
        ):
            # zx.T resident in SBUF, t-major: col = t*128 + a*32 + q*8 + b
            zxT = persist_pool.tile([P, T * FB], bf16)
            # step view for the scan's identity matmul
            zxs = zxT.rearrange("p (t a q b) -> p t a q b", t=T, a=4, q=4)
            # copy-out view for phase A: [p, a, q, b, t]
            zxc = zxT.rearrange("p (t a q b) -> p a q b t", t=T, a=4, q=4)
            b_sb = persist_pool.tile([P, M], f32)
            nc.sync.dma_start(b_sb[:], b_h.ap().rearrange("(m p) -> p m", p=P))
            # double the g-gate bias (original m-tiles 8..11)
            nc.vector.tensor_scalar_mul(b_sb[:, 8:12], b_sb[:, 8:12], 2.0)
            ident = persist_pool.tile([P, P], bf16)
            masks.make_identity(nc, ident[:])
            ident = persist_pool.tile([P, P], bf16)
            masks.make_identity(nc, ident[:])

            wk_sb = persist_pool.tile([P, KD * G], bf16)
            wr_sb = persist_pool.tile([P, KU * G], bf16)
            with tc.tile_pool(name="stage", bufs=2) as stage_pool:
                load_weight_bf16(wk_sb, wk_h, stage_pool)
                load_weight_bf16(wr_sb, wr_h, stage_pool)

            with (
                tc.tile_pool(name="nat", bufs=2) as nat_pool,
                tc.tile_pool(name="xtb", bufs=2) as xtb_pool,
                tc.tile_pool(name="gemm_psum", bufs=2, space="PSUM") as gps_pool,
                tc.tile_pool(name="state", bufs=1) as st_pool,
                tc.tile_pool(name="gates", bufs=2) as gate_pool,
                tc.tile_pool(name="tmp", bufs=2) as tmp_pool,
                tc.tile_pool(name="scan_psum", bufs=2, space="PSUM") as sps_pool,
            ):
                # ---------- phase-A work items (per chunk) ----------
                # pipeline state for the chunk currently being produced
                pa = {"nat": [None] * 4, "natb": [None] * 4,
                      "xtb": [None] * 4, "gps": None, "mms": []}

                def pa_load(cc, bp):
                    t0 = cc * TC
                    nat = nat_pool.tile([P, D], f32, name="nat", tag=f"nat{bp}")
                    for j in range(2):
                        nc.gpsimd.dma_start(
                            nat[j * TC:(j + 1) * TC, :],
                            x_ap[2 * bp + j, t0:t0 + TC, :],
                        )
                    pa["nat"][bp] = nat

                def pa_cast(bp):
                    # bf16 cast on Pool (SBUF->SBUF; keeps ACT/DVE free).
                    # Emitted several steps after the DMA so the Pool FIFO
                    # never blocks on it in front of the scan's f*c.
                    natb = nat_pool.tile([P, D], bf16, name="natb",
                                         tag=f"natb{bp}")
                    nc.gpsimd.tensor_copy(natb[:], pa["nat"][bp][:])
                    pa["natb"][bp] = natb

                def pa_transpose(idx):
                    # one xbar transpose: xtb[k] cols = bp*128 (b-major)
                    k, bp = idx // 4, idx % 4
                    if bp == 0:
                        pa["xtb"][k] = xtb_pool.tile(
                            [P, TC * B_LOC], bf16, name=f"xtb{k}",
                            tag=f"xtb{k}")
                    nc.sync.dma_start(
                        pa["xtb"][k][:, bp * P:(bp + 1) * P],
                        pa["natb"][bp][:, k * P:(k + 1) * P],
                        transpose=True,
                    )

                def pa_mm(m, kpair):
                    if kpair == 0:
                        pa["gps"] = gps_pool.tile([P, TC * B_LOC], f32,
                                                  name="gps", tag="gps")
                    ps = pa["gps"]
                    for k in (2 * kpair, 2 * kpair + 1):
                        pa["mms"].append(nc.tensor.matmul(
                            ps[:],
                            wk_sb[:, (k * M + m) * P:(k * M + m + 1) * P],
                            pa["xtb"][k][:],
                            start=(k == 0),
                            stop=(k == KD - 1),
                        ))

                def pa_copyout(cc, m):
                    t0 = cc * TC
                    dst = zxc[:, m // 4, m % 4, :, t0:t0 + TC]
                    src = pa["gps"].rearrange("p (b t) -> p b t", t=TC)[:]
                    bias = b_sb[:, m:m + 1]
                    if m % 2 == 0:
                        nc.scalar.activation(dst, src, AF.Identity, bias=bias)
                    else:
                        nc.vector.tensor_scalar(dst, src, bias, None,
                                                AluOpType.add)

                def pa_quantum(cc, s):
                    """Emit chunk cc's work quantum for sub-step s (0..63)."""
                    if s < 4:
                        pa_load(cc, s)
                    elif s < 8:
                        pass  # DMA in flight
                    elif s < 12:
                        pa_cast(s - 8)
                    elif s < 20:
                        pa_transpose(2 * (s - 12))
                        pa_transpose(2 * (s - 12) + 1)
                    elif s < 52:
                        i, phase = divmod(s - 20, 2)
                        if phase == 0 and i > 0:
                            pa_copyout(cc, i - 1)
                        pa_mm(i, phase)
                    elif s == 52:
                        pa_copyout(cc, M - 1)

                # ---------- prolog: chunk 0, densely ----------
                for bp in range(4):
                    pa_load(0, bp)
                for bp in range(4):
                    pa_cast(bp)
                for idx in range(16):
                    pa_transpose(idx)
                for m in range(M):
                    pa_mm(m, 0)
                    pa_mm(m, 1)
                    pa_copyout(0, m)

                # ---------- the scan, with chunk c+1 interleaved ----------
                hs = [[st_pool.tile([P, 2 * B_LOC], bf16, name=f"h{i}{j}")
                       for j in range(2)] for i in range(2)]
                cs = [[st_pool.tile([P, 2 * B_LOC], f32, name=f"c{i}{j}")
                       for j in range(2)] for i in range(2)]
                for j in range(2):
                    nc.vector.memset(hs[0][j][:], 0.0)
                    nc.vector.memset(cs[0][j][:], 0.0)
                hf = st_pool.tile([P, KU * B_LOC], f32, name="hf")

                def mm_dst(ps, m, half):
                    a, q = m // 4, m % 4 - 2 * half
                    return ps[:, a * 16 + q * 8:a * 16 + q * 8 + 8]

                def alloc_pss():
                    return [sps_pool.tile([P, HB], f32, name=f"ps{hf_}",
                                          tag=f"ps{hf_}") for hf_ in range(2)]

                def emit_imm(pss_t, t):
                    """zx(t) -> psum via identity matmul (start=True resets
                    the accumulators, so the weight MMs that follow are
                    order-independent).  Returns the two instructions."""
                    imms = []
                    for half in range(2):
                        imms.append(nc.tensor.matmul(
                            pss_t[half].rearrange("p (a q b) -> p a q b",
                                                  a=4, q=2)[:],
                            ident[:],
                            zxs[:, t, :, 2 * half:2 * half + 2, :],
                            start=True,
                            stop=False,
                            skip_group_check=True,
                        ))
                    return imms

                pss = alloc_pss()
                imms = emit_imm(pss, 0)
                for t in range(time_steps):
                    pp = t % 2
                    qq = 1 - pp
                    h_prev = hs[pp]
                    # Weight MM burst.  Block order (explicit dep edges pin
                    # the scheduler): psum1 i,f,g tiles; psum1 o tiles;
                    # psum0 i,f,g; psum0 o.  Half 1's critical sigmoid
                    # depends only on the first 24 pairs (instr 48, a PE
                    # hw-decode batch boundary, so its semaphore posts
                    # promptly).  Within each block, kk 2,3 (reading h half
                    # 1, which each tail finishes first) precede kk 0,1.
                    prev_block_last = None
                    for half in (1, 0):
                        for ms_group in (IFG_MS[half], O_MS[half]):
                            block_first = None
                            for kpair in (1, 0):
                                for m in ms_group:
                                    dst = mm_dst(pss[half], m, half)
                                    for kk in (2 * kpair, 2 * kpair + 1):
                                        i_mm = nc.tensor.matmul(
                                            dst,
                                            wr_sb[:, (kk * M + m) * P:
                                                  (kk * M + m + 1) * P],
                                            h_prev[kk // 2][:,
                                                (kk % 2) * B_LOC:
                                                (kk % 2 + 1) * B_LOC],
                                            start=False,
                                            stop=(ms_group is O_MS[half]
                                                  and kpair == 0 and kk == 1
                                                  and m == ms_group[-1]),
                                            skip_group_check=True,
                                        )
                                        if block_first is None:
                                            block_first = i_mm
                                        block_last = i_mm
                            if prev_block_last is not None:
                                add_dep_helper(
                                    block_first.ins, prev_block_last.ins,
                                    reason="pin scan MM block order")
                            prev_block_last = block_last
                    # next step's identity matmuls ride at the end of this
                    # step's PE stream (they execute during the tails)
                    last_t = t == time_steps - 1
                    if not last_t:
                        pss_next = alloc_pss()
                        imms_next = emit_imm(pss_next, t + 1)
                        add_dep_helper(imms_next[0].ins, prev_block_last.ins,
                                       reason="imm after burst")
                        add_dep_helper(imms_next[1].ins, prev_block_last.ins,
                                       reason="imm after burst")
                    # phase-A quantum for the next chunk; its PE matmuls are
                    # pinned behind the identity matmuls so they run in the
                    # tail's idle window, not inside the critical burst.
                    # (All its copy-outs are emitted by sub-step 52, well
                    # before the step-64c identity matmul that reads them.)
                    cc, s = divmod(t, TC)
                    if cc + 1 < NCH:
                        pa["mms"] = []
                        pa_quantum(cc + 1, s)
                        for i_pamm in pa["mms"]:
                            add_dep_helper(i_pamm.ins, prev_block_last.ins,
                                           reason="pa mm after burst")
                    # tails; full half-1 chain first so its tanh_c isn't
                    # queued behind half-0's sigmoids on the ACT engine (an
                    # explicit dep pins that order); f*c runs on Pool
                    # concurrently with the DVE's t2.
                    prev_tc = None
                    for half in (1, 0):
                        gt = gate_pool.tile([P, HB], f32, name=f"gt{half}",
                                            tag=f"gt{half}")
                        # i,f sigmoids and tanh g = 2*sigm(2 z_g)-1 (g
                        # columns pre-scaled by 2); o separately, off the
                        # critical chain
                        i_sig = nc.scalar.activation(gt[:, 0:48],
                                                     pss[half][:, 0:48],
                                                     AF.Sigmoid)
                        nc.scalar.activation(gt[:, 48:64], pss[half][:, 48:64],
                                             AF.Sigmoid)
                        if prev_tc is not None:
                            add_dep_helper(i_sig.ins, prev_tc.ins,
                                           reason="sig0 after tanh_c1 on ACT")
                        t1 = tmp_pool.tile([P, 2 * B_LOC], f32,
                                           name=f"t1{half}", tag=f"t1{half}")
                        nc.gpsimd.tensor_mul(t1[:], gt[:, 16:32],
                                             cs[pp][half][:])
                        t2 = tmp_pool.tile([P, 2 * B_LOC], f32,
                                           name=f"t2{half}", tag=f"t2{half}")
                        # t2 = (sig_g - 0.5) * i   [= i * tanh(g) / 2]
                        nc.vector.scalar_tensor_tensor(
                            t2[:], gt[:, 32:48], 0.5, gt[:, 0:16],
                            AluOpType.subtract, AluOpType.mult,
                        )
                        # c' = 2*t2 + t1
                        nc.vector.scalar_tensor_tensor(
                            cs[qq][half][:], t2[:], 2.0, t1[:],
                            AluOpType.mult, AluOpType.add,
                        )
                        tc_t = tmp_pool.tile([P, 2 * B_LOC], f32,
                                             name=f"tc{half}", tag=f"tc{half}")
                        prev_tc = nc.scalar.activation(tc_t[:],
                                                       cs[qq][half][:],
                                                       AF.Tanh)
                        if last_t:
                            nc.vector.tensor_mul(
                                hf[:, half * 16:(half + 1) * 16],
                                gt[:, 48:64], tc_t[:],
                            )
                        else:
                            nc.vector.tensor_mul(hs[qq][half][:],
                                                 gt[:, 48:64], tc_t[:])
                    if not last_t:
                        pss = pss_next
                        imms = imms_next

                for kk in range(KU):
                    nc.sync.dma_start(
                        out_h.ap()[:, kk * P:(kk + 1) * P].rearrange("b p -> p b"),
                        hf[:, kk * B_LOC:(kk + 1) * B_LOC],
                    )

    nc.compile()
    return nc


def _get_nc(time_steps=T):
    key = time_steps
    if key not in _CACHE:
        _CACHE[key] = _build(time_steps)
    return _CACHE[key]


def kernel(x, Wk, Wr, b):
    from concourse import bass_utils

    x = np.ascontiguousarray(np.asarray(x, dtype=np.float32))
    Wk = np.ascontiguousarray(np.asarray(Wk, dtype=np.float32))
    Wr = np.ascontiguousarray(np.asarray(Wr, dtype=np.float32))
    b = np.ascontiguousarray(np.asarray(b, dtype=np.float32))

    nc = _get_nc(T)
    in_maps = [
        {
            "x": x[c * B_LOC:(c + 1) * B_LOC],
            "Wk": Wk,
            "Wr": Wr,
            "b": b,
        }
        for c in range(N_CORES)
    ]
    res = bass_utils.run_bass_kernel_spmd(nc, in_maps, core_ids=list(range(N_CORES)))
    return np.concatenate([res.results[c]["h_last"] for c in range(N_CORES)], axis=0)


# revision 19
# speedup vs baseline: 1.0444x; 1.0444x over previous
"""Trainium2 Bass kernel for BasicLSTM (B=64, T=512, D=U=512).

Sharding: data-parallel over batch across 8 cores (8 rows/core), weights
replicated; the sequential time scan runs locally per core.

Per-core strategy (everything unit-major / "transposed", all-SBUF):
  zx.T = Wk.T @ x.T + b is computed in 64-step chunks that OVERLAP the
  recurrent scan: while the scan consumes chunk c, the input projection
  for chunk c+1 streams in (DMA loads + bf16 casts on Pool + DMA-xbar
  transposes) and its matmuls/copy-outs slot into the PE/ACT/DVE idle
  windows of the scan steps.  zx.T lives in SBUF bf16, t-major
  (col = t*128 + a*32 + q*8 + b), so each scan step reads one contiguous
  128-col block and phase-A chunks write disjoint ranges.
  The g-gate columns of Wk/Wr/b are pre-scaled by 2 so the scan evaluates
  all four gates with a single sigmoid (tanh(x) = 2*sigm(2x)-1).

  Scan step (per core, batch 8, unit-major):
    psum[half] = zx.T[t] (identity matmul) + sum_k Wr[k,m].T @ h.T[k]
    s = sigmoid(psum)            (one ACT op per half)
    t2 = (s_g - 0.5) * s_i ; c' = 2*t2 + f*c   (fused DVE STT ops)
    h' = s_o * tanh(c')          (ACT + DVE)
  Psum half 1's 32 matmuls are issued first so its sigmoid fires
  mid-burst and its tail overlaps the half-0 matmuls; kk in {2,3}
  (reading h half 1, which each tail finishes first) precede kk in {0,1}.
  h is bf16 (feeds the next matmul), c stays fp32.
"""

import numpy as np

B, T, D, U = 64, 512, 512, 512
G = 4 * U            # gates
P = 128              # partitions
N_CORES = 8
B_LOC = B // N_CORES  # 8
KD = D // P          # 4 k-tiles for x@Wk
KU = U // P          # 4 k-tiles for h@Wr
M = G // P           # 16 m-tiles of gates
TC = 64              # timesteps per phase-A chunk
NCH = T // TC        # 8 chunks
FB = M * B_LOC       # 128 free cols of z per step
HB = FB // 2         # 64 cols per half

# natural gate order [i, f, g, o]; a = m//4 is the gate class
# halves: half h holds m-tiles {4a + q : a in 0..3} for q in {2h, 2h+1}
HALF_MS = [[0, 4, 8, 12, 1, 5, 9, 13], [2, 6, 10, 14, 3, 7, 11, 15]]
# the critical chain needs only i,f,g (classes 0..2); o feeds just h'=o*tanh(c)
IFG_MS = [[m for m in ms if m < 12] for ms in HALF_MS]
O_MS = [[m for m in ms if m >= 12] for ms in HALF_MS]

_CACHE = {}


def _build(time_steps=T):
    import concourse.bacc as bacc
    import concourse.tile as tile
    import concourse.mybir as mybir
    from concourse import masks
    from concourse import masks
    from concourse.alu_op_type import AluOpType
    from bass_rust import add_dep_helper

    f32 = mybir.dt.float32
    bf16 = mybir.dt.bfloat16
    AF = mybir.ActivationFunctionType

    nc = bacc.Bacc(
        "TRN2",
        target_bir_lowering=False,
        debug=False,
        enable_asserts=True,
        num_devices=N_CORES,
    )

    x_h = nc.dram_tensor("x", [B_LOC, T, D], f32, kind="ExternalInput")
    wk_h = nc.dram_tensor("Wk", [D, G], f32, kind="ExternalInput")
    wr_h = nc.dram_tensor("Wr", [U, G], f32, kind="ExternalInput")
    b_h = nc.dram_tensor("b", [G], f32, kind="ExternalInput")
    out_h = nc.dram_tensor("h_last", [B_LOC, U], f32, kind="ExternalOutput")

    x_ap = x_h.ap()

    def load_weight_bf16(dst, src_h, stage_pool):
        """[512, 2048] fp32 weight -> dst bf16 [128, 64*128] laid out as
        (k, new_m) tiles of [128, 128] with the [i,f,o,g] gate reorder.
        The g tiles (new m 12..15) are scaled by 2 for the sigmoid-only
        gate evaluation."""
        for k in range(KD):
            st = stage_pool.tile([P, G], f32, name="wstage", tag="wstage")
            nc.sync.dma_start(st[:], src_h.ap()[k * P:(k + 1) * P, :])
            # split the big casts between DVE and Pool so the prolog isn't
            # serialized on one engine
            for eng, nm0, w in ((nc.vector, 0, 4), (nc.gpsimd, 4, 4),
                                (nc.vector, 12, 2), (nc.gpsimd, 14, 2)):
                eng.tensor_copy(
                    dst[:, (k * M + nm0) * P:(k * M + nm0 + w) * P],
                    st[:, nm0 * P:(nm0 + w) * P],
                )
            nc.vector.tensor_scalar_mul(
                dst[:, (k * M + 8) * P:(k * M + 10) * P],
                st[:, 8 * P:10 * P],
                2.0,
            )
            nc.gpsimd.tensor_scalar_mul(
                dst[:, (k * M + 10) * P:(k * M + 12) * P],
                st[:, 10 * P:12 * P],
                2.0,
            )

    with tile.TileContext(nc) as tc:
        with (
            tc.tile_pool(name="persist", bufs=1) as persist_pool,
        ):
            # zx.T resident in SBUF, t-major: col = t*128 + a*32 + q*8 + b
            zxT = persist_pool.tile([P, T * FB], bf16)
            # step view for the scan's identity matmul
            zxs = zxT.rearrange("p (t a q b) -> p t a q b", t=T, a=4, q=4)
            # copy-out view for phase A: [p, a, q, b, t]
            zxc = zxT.rearrange("p (t a q b) -> p a q b t", t=T, a=4, q=4)
            b_sb = persist_pool.tile([P, M], f32)
            nc.sync.dma_start(b_sb[:], b_h.ap().rearrange("(m p) -> p m", p=P))
            # double the g-gate bias (original m-tiles 8..11)
            nc.vector.tensor_scalar_mul(b_sb[:, 8:12], b_sb[:, 8:12], 2.0)
            ident = persist_pool.tile([P, P], bf16)
            masks.make_identity(nc, ident[:])
            ident = persist_pool.tile([P, P], bf16)
            masks.make_identity(nc, ident[:])

            wk_sb = persist_pool.tile([P, KD * G], bf16)
            wr_sb = persist_pool.tile([P, KU * G], bf16)
            with tc.tile_pool(name="stage", bufs=2) as stage_pool:
                load_weight_bf16(wk_sb, wk_h, stage_pool)
                load_weight_bf16(wr_sb, wr_h, stage_pool)

            with (
                tc.tile_pool(name="nat", bufs=2) as nat_pool,
                tc.tile_pool(name="xtb", bufs=2) as xtb_pool,
                tc.tile_pool(name="gemm_psum", bufs=2, space="PSUM") as gps_pool,
                tc.tile_pool(name="state", bufs=1) as st_pool,
                tc.tile_pool(name="gates", bufs=2) as gate_pool,
                tc.tile_pool(name="tmp", bufs=2) as tmp_pool,
                tc.tile_pool(name="scan_psum", bufs=2, space="PSUM") as sps_pool,
            ):
                # ---------- phase-A work items (per chunk) ----------
                # pipeline state for the chunk currently being produced
                pa = {"nat": [None] * 4, "natb": [None] * 4,
                      "xtb": [None] * 4, "gps": None, "mms": []}

                def pa_load(cc, bp):
                    t0 = cc * TC
                    nat = nat_pool.tile([P, D], f32, name="nat", tag=f"nat{bp}")
                    for j in range(2):
                        nc.gpsimd.dma_start(
                            nat[j * TC:(j + 1) * TC, :],
                            x_ap[2 * bp + j, t0:t0 + TC, :],
                        )
                    pa["nat"][bp] = nat

                def pa_cast(bp):
                    # bf16 cast on Pool (SBUF->SBUF; keeps ACT/DVE free).
                    # Emitted several steps after the DMA so the Pool FIFO
                    # never blocks on it in front of the scan's f*c.
                    natb = nat_pool.tile([P, D], bf16, name="natb",
                                         tag=f"natb{bp}")
                    nc.gpsimd.tensor_copy(natb[:], pa["nat"][bp][:])
                    pa["natb"][bp] = natb

                def pa_transpose(idx):
                    # one xbar transpose: xtb[k] cols = bp*128 (b-major)
                    k, bp = idx // 4, idx % 4
                    if bp == 0:
                        pa["xtb"][k] = xtb_pool.tile(
                            [P, TC * B_LOC], bf16, name=f"xtb{k}",
                            tag=f"xtb{k}")
                    nc.sync.dma_start(
                        pa["xtb"][k][:, bp * P:(bp + 1) * P],
                        pa["natb"][bp][:, k * P:(k + 1) * P],
                        transpose=True,
                    )

                def pa_mm(m, kpair):
                    if kpair == 0:
                        pa["gps"] = gps_pool.tile([P, TC * B_LOC], f32,
                                                  name="gps", tag="gps")
                    ps = pa["gps"]
                    for k in (2 * kpair, 2 * kpair + 1):
                        pa["mms"].append(nc.tensor.matmul(
                            ps[:],
                            wk_sb[:, (k * M + m) * P:(k * M + m + 1) * P],
                            pa["xtb"][k][:],
                            start=(k == 0),
                            stop=(k == KD - 1),
                        ))

                def pa_copyout(cc, m):
                    t0 = cc * TC
                    dst = zxc[:, m // 4, m % 4, :, t0:t0 + TC]
                    src = pa["gps"].rearrange("p (b t) -> p b t", t=TC)[:]
                    bias = b_sb[:, m:m + 1]
                    if m % 2 == 0:
                        nc.scalar.activation(dst, src, AF.Identity, bias=bias)
                    else:
                        nc.vector.tensor_scalar(dst, src, bias, None,
                                                AluOpType.add)

                def pa_quantum(cc, s):
                    """Emit chunk cc's work quantum for sub-step s (0..63)."""
                    if s < 4:
                        pa_load(cc, s)
                    elif s < 8:
                        pass  # DMA in flight
                    elif s < 12:
                        pa_cast(s - 8)
                    elif s < 20:
                        pa_transpose(2 * (s - 12))
                        pa_transpose(2 * (s - 12) + 1)
                    elif s < 52:
                        i, phase = divmod(s - 20, 2)
                        if phase == 0 and i > 0:
                            pa_copyout(cc, i - 1)
                        pa_mm(i, phase)
                    elif s == 52:
                        pa_copyout(cc, M - 1)

                # ---------- prolog: chunk 0, densely ----------
                for bp in range(4):
                    pa_load(0, bp)
                for bp in range(4):
                    pa_cast(bp)
                for idx in range(16):
                    pa_transpose(idx)
                for m in range(M):
                    pa_mm(m, 0)
                    pa_mm(m, 1)
                    pa_copyout(0, m)

                # ---------- the scan, with chunk c+1 interleaved ----------
                hs = [[st_pool.tile([P, 2 * B_LOC], bf16, name=f"h{i}{j}")
                       for j in range(2)] for i in range(2)]
                cs = [[st_pool.tile([P, 2 * B_LOC], f32, name=f"c{i}{j}")
                       for j in range(2)] for i in range(2)]
                for j in range(2):
                    nc.vector.memset(hs[0][j][:], 0.0)
                    nc.vector.memset(cs[0][j][:], 0.0)
                hf = st_pool.tile([P, KU * B_LOC], f32, name="hf")

                def mm_dst(ps, m, half):
                    a, q = m // 4, m % 4 - 2 * half
                    return ps[:, a * 16 + q * 8:a * 16 + q * 8 + 8]

                def alloc_pss():
                    return [sps_pool.tile([P, HB], f32, name=f"ps{hf_}",
                                          tag=f"ps{hf_}") for hf_ in range(2)]

                def emit_imm(pss_t, t):
                    """zx(t) -> psum via identity matmul (start=True resets
                    the accumulators, so the weight MMs that follow are
                    order-independent).  Returns the two instructions."""
                    imms = []
                    for half in range(2):
                        imms.append(nc.tensor.matmul(
                            pss_t[half].rearrange("p (a q b) -> p a q b",
                                                  a=4, q=2)[:],
                            ident[:],
                            zxs[:, t, :, 2 * half:2 * half + 2, :],
                            start=True,
                            stop=False,
                            skip_group_check=True,
                        ))
                    return imms

                pss = alloc_pss()
                imms = emit_imm(pss, 0)
                for t in range(time_steps):
                    pp = t % 2
                    qq = 1 - pp
                    h_prev = hs[pp]
                    # Weight MM burst, grouped by gating: all kk 2,3
                    # pairs (waiting on h half 1, which each tail finishes
                    # first) run first, then all kk 0,1 pairs (h half 0).
                    # Within each gating class, psum 1's i,f,g tiles lead so
                    # both sigmas' dependencies clear within ~12 pairs of
                    # h half 0 landing.  Explicit dep edges pin the order.
                    prev_block_last = None
                    for kpair in (1, 0):
                        for half in (1, 0):
                            for ms_group in (IFG_MS[half], O_MS[half]):
                                block_first = None
                                for m in ms_group:
                                    dst = mm_dst(pss[half], m, half)
                                    for kk in (2 * kpair, 2 * kpair + 1):
                                        i_mm = nc.tensor.matmul(
                                            dst,
                                            wr_sb[:, (kk * M + m) * P:
                                                  (kk * M + m + 1) * P],
                                            h_prev[kk // 2][:,
                                                (kk % 2) * B_LOC:
                                                (kk % 2 + 1) * B_LOC],
                                            start=False,
                                            stop=(kpair == 0 and kk == 1
                                                  and ms_group is O_MS[half]
                                                  and m == ms_group[-1]),
                                            skip_group_check=True,
                                        )
                                        if block_first is None:
                                            block_first = i_mm
                                        block_last = i_mm
                                if prev_block_last is not None:
                                    add_dep_helper(
                                        block_first.ins, prev_block_last.ins,
                                        reason="pin scan MM block order")
                                prev_block_last = block_last
                    # next step's identity matmuls ride at the end of this
                    # step's PE stream (they execute during the tails)
                    last_t = t == time_steps - 1
                    if not last_t:
                        pss_next = alloc_pss()
                        imms_next = emit_imm(pss_next, t + 1)
                        add_dep_helper(imms_next[0].ins, prev_block_last.ins,
                                       reason="imm after burst")
                        add_dep_helper(imms_next[1].ins, prev_block_last.ins,
                                       reason="imm after burst")
                    # phase-A quantum for the next chunk; its PE matmuls are
                    # pinned behind the identity matmuls so they run in the
                    # tail's idle window, not inside the critical burst.
                    # (All its copy-outs are emitted by sub-step 52, well
                    # before the step-64c identity matmul that reads them.)
                    cc, s = divmod(t, TC)
                    if cc + 1 < NCH:
                        pa["mms"] = []
                        pa_quantum(cc + 1, s)
                        for i_pamm in pa["mms"]:
                            add_dep_helper(i_pamm.ins, prev_block_last.ins,
                                           reason="pa mm after burst")
                    # tails; full half-1 chain first so its tanh_c isn't
                    # queued behind half-0's sigmoids on the ACT engine (an
                    # explicit dep pins that order); f*c runs on Pool
                    # concurrently with the DVE's t2.
                    prev_tc = None
                    for half in (1, 0):
                        gt = gate_pool.tile([P, HB], f32, name=f"gt{half}",
                                            tag=f"gt{half}")
                        # i,f sigmoids and tanh g = 2*sigm(2 z_g)-1 (g
                        # columns pre-scaled by 2); o separately, off the
                        # critical chain
                        i_sig = nc.scalar.activation(gt[:, 0:48],
                                                     pss[half][:, 0:48],
                                                     AF.Sigmoid)
                        nc.scalar.activation(gt[:, 48:64], pss[half][:, 48:64],
                                             AF.Sigmoid)
                        if prev_tc is not None:
                            add_dep_helper(i_sig.ins, prev_tc.ins,
                                           reason="sig0 after tanh_c1 on ACT")
                        t1 = tmp_pool.tile([P, 2 * B_LOC], f32,
                                           name=f"t1{half}", tag=f"t1{half}")
                        nc.gpsimd.tensor_mul(t1[:], gt[:, 16:32],
                                             cs[pp][half][:])
                        t2 = tmp_pool.tile([P, 2 * B_LOC], f32,
                                           name=f"t2{half}", tag=f"t2{half}")
                        # t2 = (sig_g - 0.5) * i   [= i * tanh(g) / 2]
                        nc.vector.scalar_tensor_tensor(
                            t2[:], gt[:, 32:48], 0.5, gt[:, 0:16],
                            AluOpType.subtract, AluOpType.mult,
                        )
                        # c' = 2*t2 + t1
                        nc.vector.scalar_tensor_tensor(
                            cs[qq][half][:], t2[:], 2.0, t1[:],
                            AluOpType.mult, AluOpType.add,
                        )
                        tc_t = tmp_pool.tile([P, 2 * B_LOC], f32,
                                             name=f"tc{half}", tag=f"tc{half}")
                        prev_tc = nc.scalar.activation(tc_t[:],
                                                       cs[qq][half][:],
                                                       AF.Tanh)
                        if last_t:
                            nc.vector.tensor_mul(
                                hf[:, half * 16:(half + 1) * 16],
                                gt[:, 48:64], tc_t[:],
                            )
                        else:
                            nc.vector.tensor_mul(hs[qq][half][:],
                                                 gt[:, 48:64], tc_t[:])
                    if not last_t:
                        pss = pss_next
                        imms = imms_next

                for kk in range(KU):
                    nc.sync.dma_start(
                        out_h.ap()[:, kk * P:(kk + 1) * P].rearrange("b p -> p b"),
                        hf[:, kk * B_LOC:(kk + 1) * B_LOC],
                    )

    nc.compile()
    return nc


def _get_nc(time_steps=T):
    key = time_steps
    if key not in _CACHE:
        _CACHE[key] = _build(time_steps)
    return _CACHE[key]


def kernel(x, Wk, Wr, b):
    from concourse import bass_utils

    x = np.ascontiguousarray(np.asarray(x, dtype=np.float32))
    Wk = np.ascontiguousarray(np.asarray(Wk, dtype=np.float32))
    Wr = np.ascontiguousarray(np.asarray(Wr, dtype=np.float32))
    b = np.ascontiguousarray(np.asarray(b, dtype=np.float32))

    nc = _get_nc(T)
    in_maps = [
        {
            "x": x[c * B_LOC:(c + 1) * B_LOC],
            "Wk": Wk,
            "Wr": Wr,
            "b": b,
        }
        for c in range(N_CORES)
    ]
    res = bass_utils.run_bass_kernel_spmd(nc, in_maps, core_ids=list(range(N_CORES)))
    return np.concatenate([res.results[c]["h_last"] for c in range(N_CORES)], axis=0)
